# revision 20
# baseline (speedup 1.0000x reference)
"""Performer self-attention (B=4, S=4096, D=1024, H=16) on 8 TRN2 NeuronCores.

Math (per token, per head):
  q = exp(-0.5*(x@Wq+bq)^2); k likewise; v = x@Wv+bv
  coeff = sum_hd(q*k);  out = (coeff*v) @ Wo + bo

Sharding: data-parallel over the 16384 token rows -> 2048 rows per core.
Each core runs the same Bass program on its row slab with all weights
resident in SBUF.

Kernel design (per 128-token tile, token-major):
  - x^T obtained via 8 PE transposes (f32r; inputs get rounded to f32r by
    the matmul anyway)
  - q/k/v projections as float32r matmuls (full rate at N=512, ~1.6e-4
    rounding vs 2.3e-3 for bf16); q/k/v biases folded in via K=1 ones-row
    matmuls, the output bias via a broadcast tensor_add during evacuation
  - feature map on ScalarE (Square, then Exp with scale=-0.5; both live in
    the exp_and_others ACT table set -> one table load)
  - per-head reduction + coeff*v broadcast on VectorE (step-0 free-dim AP)
  - qkv transposed back via PE, output projection + bias, ScalarE evac, DMA
  - two-stage software pipeline: tile t's transposes+projections are
    emitted before tile t-1's elementwise+output stage so the PE never
    waits on the ACT/DVE chain
  - DMAs split across queues: x tiles + Wk/Wv chunks on the Scalar HWDGE
    queue, Wq/biases/Wo on Sync, so the first tiles start immediately
"""
import numpy as np
import concourse.bass as bass
import concourse.tile as tile
from concourse import bacc, mybir
import concourse.bass_utils as bass_utils
from concourse.masks import make_identity

F32 = mybir.dt.float32
F32R = mybir.dt.float32r

B, S, D = 4, 4096, 1024
H = 16
HD = D // H
NCORES = 8
ROWS = B * S // NCORES  # 2048 rows per core
NCHUNK = D // 128  # 8 contraction chunks


def _build(rows=ROWS, num_devices=NCORES):
    ntiles = rows // 128
    nc = bacc.Bacc("TRN2", target_bir_lowering=False, debug=False,
                   num_devices=num_devices)
    X = nc.dram_tensor("x", [rows, D], F32R, kind="ExternalInput").ap()
    Wq = nc.dram_tensor("Wq", [D, D], F32R, kind="ExternalInput").ap()
    Wk = nc.dram_tensor("Wk", [D, D], F32R, kind="ExternalInput").ap()
    Wv = nc.dram_tensor("Wv", [D, D], F32R, kind="ExternalInput").ap()
    Wo = nc.dram_tensor("Wo", [D, D], F32R, kind="ExternalInput").ap()
    BQ = nc.dram_tensor("bq", [D], F32R, kind="ExternalInput").ap()
    BK = nc.dram_tensor("bk", [D], F32R, kind="ExternalInput").ap()
    BV = nc.dram_tensor("bv", [D], F32R, kind="ExternalInput").ap()
    BO = nc.dram_tensor("bo", [D], F32R, kind="ExternalInput").ap()
    OUT = nc.dram_tensor("out", [rows, D], F32, kind="ExternalOutput").ap()

    with tile.TileContext(nc) as tc:
        with tc.tile_pool(name="wpool", bufs=1) as wpool, \
             tc.tile_pool(name="cpool", bufs=1) as cpool, \
             tc.tile_pool(name="xpool", bufs=3) as xpool, \
             tc.tile_pool(name="xtp", bufs=2) as xtp, \
             tc.tile_pool(name="ew", bufs=2) as ew, \
             tc.tile_pool(name="qkvp", bufs=2) as qkvp, \
             tc.tile_pool(name="outp", bufs=2) as outp, \
             tc.tile_pool(name="ps", bufs=8, space="PSUM") as ps:

            # ---- one-time setup ----
            ident_f32 = cpool.tile([128, 128], F32, tag="ident_f32")
            make_identity(nc, ident_f32)
            ident = cpool.tile([128, 128], F32R, tag="ident")
            nc.vector.tensor_copy(ident, ident_f32)

            ones_f32 = cpool.tile([1, 128], F32, tag="ones_f32")
            nc.vector.memset(ones_f32, 1.0)
            ones = cpool.tile([1, 128], F32R, tag="ones")
            nc.vector.tensor_copy(ones, ones_f32)

            # biases packed on partition 0: b_sb[0, w, :] = bias of weight w
            b_sb = cpool.tile([1, 3, D], F32R, tag="b_sb")
            bo_bc = cpool.tile([128, D], F32R, tag="bo_bc")

            # weights: w_sb[w] is [128, NCHUNK, D]; chunk c = W[c*128:(c+1)*128]
            w_sb = []
            for w in range(4):
                t = wpool.tile([128, NCHUNK, D], F32R, tag=f"w{w}",
                               name=f"w{w}")
                w_sb.append(t)
            for c in range(NCHUNK):
                nc.sync.dma_start(out=w_sb[0][:, c, :],
                                  in_=Wq[c * 128:(c + 1) * 128, :])
            for i, Bi in enumerate([BQ, BK, BV]):
                nc.sync.dma_start(out=b_sb[:, i, :],
                                  in_=Bi.rearrange("(a n) -> a n", a=1))
            nc.sync.dma_start(
                out=bo_bc,
                in_=bass.AP(tensor=BO.tensor, offset=BO.offset,
                            ap=[[0, 128], [BO.ap[0][0], D]]))
            for c in range(NCHUNK):
                nc.sync.dma_start(out=w_sb[3][:, c, :],
                                  in_=Wo[c * 128:(c + 1) * 128, :])

            def dma_weights_kv():
                for c in range(NCHUNK):
                    for w, W in [(1, Wk), (2, Wv)]:
                        nc.scalar.dma_start(out=w_sb[w][:, c, :],
                                            in_=W[c * 128:(c + 1) * 128, :])

            x_tiles = [None] * ntiles

            def dma_x(t):
                x_sb = xpool.tile([128, D], F32R, tag="x", name="x_sb")
                nc.scalar.dma_start(out=x_sb, in_=X[t * 128:(t + 1) * 128, :])
                x_tiles[t] = x_sb

            early = [None] * ntiles  # t -> (proj_ps[3],)
            xTs = [None] * ntiles

            def stage_transpose(t):
                x_sb = x_tiles[t]
                xT = xtp.tile([128, NCHUNK, 128], F32R, tag="xT", name="xT")
                xTf = xT.rearrange("p c t -> p (c t)")
                for g in range(2):
                    xT_ps = ps.tile([128, 512], F32R, tag="ps", name="xT_ps")
                    for cc in range(4):
                        c = g * 4 + cc
                        nc.tensor.transpose(
                            xT_ps[:, cc * 128:(cc + 1) * 128],
                            x_sb[:, c * 128:(c + 1) * 128], ident)
                    nc.vector.tensor_copy(xTf[:, g * 512:(g + 1) * 512], xT_ps)
                xTs[t] = xT

            def stage_proj(t):
                xT = xTs[t]
                proj_ps = [[ps.tile([128, 512], F32, tag="ps",
                                    name=f"proj{w}h{h}")
                            for h in range(2)] for w in range(3)]
                for c in range(NCHUNK):
                    for w in range(3):
                        for h in range(2):
                            nc.tensor.matmul(
                                proj_ps[w][h],
                                xT[:, c, :],
                                w_sb[w][:, c, h * 512:(h + 1) * 512],
                                start=(c == 0), stop=False)
                for w in range(3):
                    for h in range(2):
                        nc.tensor.matmul(
                            proj_ps[w][h],
                            ones, b_sb[:, w, h * 512:(h + 1) * 512],
                            start=False, stop=True)
                early[t] = proj_ps

            qkvTs = [None] * ntiles
            qkvs = [None] * ntiles

            def stage_mid(t):
                stage_mid_ew(t)
                stage_mid_T(t)

            def stage_mid_ew(t):
                proj_ps = early[t]
                # feature map: qe = exp(-0.5*(q+b)^2), ke likewise (ScalarE)
                sq_q = ew.tile([128, D], F32, tag="sq", name="sq_q")
                for h in range(2):
                    nc.scalar.activation(sq_q[:, h * 512:(h + 1) * 512],
                                         proj_ps[0][h],
                                         mybir.ActivationFunctionType.Square)
                qe = ew.tile([128, D], F32, tag="qe", name="qe")
                nc.scalar.activation(qe, sq_q,
                                     mybir.ActivationFunctionType.Exp,
                                     scale=-0.5)
                sq_k = ew.tile([128, D], F32, tag="sq", name="sq_k")
                for h in range(2):
                    nc.scalar.activation(sq_k[:, h * 512:(h + 1) * 512],
                                         proj_ps[1][h],
                                         mybir.ActivationFunctionType.Square)
                ke = ew.tile([128, D], F32, tag="qe", name="ke")
                nc.scalar.activation(ke, sq_k,
                                     mybir.ActivationFunctionType.Exp,
                                     scale=-0.5)

                # coeff[p, h] = sum_j qe[p,h,j]*ke[p,h,j]
                qk = ew.tile([128, D], F32, tag="sq", name="qk")
                nc.vector.tensor_mul(qk, qe, ke)
                coeff = ew.tile([128, H], F32, tag="coeff", name="coeff")
                nc.vector.reduce_sum(coeff,
                                     qk.rearrange("p (h j) -> p h j", h=H),
                                     axis=mybir.AxisListType.X)

                # qkv = coeff (broadcast over HD) * v  -> f32r for out proj
                qkv = qkvp.tile([128, D], F32R, tag="qkv", name="qkv")
                qkv3 = qkv.rearrange("p (h j) -> p h j", h=H)
                for hh in range(2):
                    coeff_b = bass.AP(
                        tensor=coeff.tensor,
                        offset=coeff.offset + hh * 8 * coeff.ap[1][0],
                        ap=[coeff.ap[0], [coeff.ap[1][0], 8], [0, HD]])
                    nc.vector.tensor_mul(
                        qkv3[:, hh * 8:(hh + 1) * 8, :],
                        proj_ps[2][hh].rearrange("p (h j) -> p h j", h=8),
                        coeff_b)

                qkvs[t] = qkv

            def stage_mid_T(t):
                qkv = qkvs[t]
                # transpose qkv
                qkvT = qkvp.tile([128, NCHUNK, 128], F32R, tag="qkvT",
                                 name="qkvT")
                qkvTf = qkvT.rearrange("p c t -> p (c t)")
                for g in range(2):
                    qkvT_ps = ps.tile([128, 512], F32R, tag="ps",
                                      name="qkvT_ps")
                    for cc in range(4):
                        c = g * 4 + cc
                        nc.tensor.transpose(
                            qkvT_ps[:, cc * 128:(cc + 1) * 128],
                            qkv[:, c * 128:(c + 1) * 128], ident)
                    nc.scalar.copy(qkvTf[:, g * 512:(g + 1) * 512], qkvT_ps)
                qkvTs[t] = qkvT

            def stage_out(t):
                rs = t * 128
                qkvT = qkvTs[t]
                # output projection + bias
                out_ps = [ps.tile([128, 512], F32, tag="ps", name=f"out_h{h}")
                          for h in range(2)]
                for c in range(NCHUNK):
                    for h in range(2):
                        nc.tensor.matmul(
                            out_ps[h],
                            qkvT[:, c, :],
                            w_sb[3][:, c, h * 512:(h + 1) * 512],
                            start=(c == 0), stop=(c == NCHUNK - 1))
                out_sb = outp.tile([128, D], F32, tag="out", name="out_sb")
                for h in range(2):
                    nc.vector.tensor_add(out_sb[:, h * 512:(h + 1) * 512],
                                         out_ps[h],
                                         bo_bc[:, h * 512:(h + 1) * 512])
                nc.scalar.dma_start(out=OUT[rs:rs + 128, :], in_=out_sb)

            # ---- pipelined emission (transpose one tile ahead) ----
            dma_x(0)
            if ntiles > 1:
                dma_x(1)
            dma_weights_kv()
            stage_transpose(0)
            for t in range(ntiles):
                if t + 2 < ntiles:
                    dma_x(t + 2)
                if t + 1 < ntiles:
                    stage_transpose(t + 1)
                stage_proj(t)
                if t >= 2:
                    stage_out(t - 2)
                if t >= 1:
                    stage_mid(t - 1)
            stage_mid_ew(ntiles - 1)
            if ntiles >= 2:
                stage_out(ntiles - 2)
            stage_mid_T(ntiles - 1)
            stage_out(ntiles - 1)

    nc.compile()
    return nc


_NC = None
LAST_RESULT = None  # BassKernelResults of the most recent run (for test.py)


def kernel(x, Wq, bq, Wk, bk, Wv, bv, Wo, bo, _trace=False):
    global _NC, LAST_RESULT
    if _NC is None:
        _NC = _build()

    x = np.ascontiguousarray(np.asarray(x, dtype=np.float32)).reshape(B * S, D)
    weights = {
        "Wq": np.ascontiguousarray(np.asarray(Wq, dtype=np.float32)),
        "Wk": np.ascontiguousarray(np.asarray(Wk, dtype=np.float32)),
        "Wv": np.ascontiguousarray(np.asarray(Wv, dtype=np.float32)),
        "Wo": np.ascontiguousarray(np.asarray(Wo, dtype=np.float32)),
        "bq": np.ascontiguousarray(np.asarray(bq, dtype=np.float32)),
        "bk": np.ascontiguousarray(np.asarray(bk, dtype=np.float32)),
        "bv": np.ascontiguousarray(np.asarray(bv, dtype=np.float32)),
        "bo": np.ascontiguousarray(np.asarray(bo, dtype=np.float32)),
    }
    in_maps = [{"x": x[i * ROWS:(i + 1) * ROWS], **weights}
               for i in range(NCORES)]
    res = bass_utils.run_bass_kernel_spmd(
        _NC, in_maps, core_ids=list(range(NCORES)), trace=_trace)
    LAST_RESULT = res
    out = np.concatenate([res.results[i]["out"] for i in range(NCORES)],
                         axis=0)
    return out.reshape(B, S, D).astype(np.float32)


# revision 23
# speedup vs baseline: 1.0664x; 1.0664x over previous
"""Performer self-attention (B=4, S=4096, D=1024, H=16) on 8 TRN2 NeuronCores.

Math (per token, per head):
  q = exp(-0.5*(x@Wq+bq)^2); k likewise; v = x@Wv+bv
  coeff = sum_hd(q*k);  out = (coeff*v) @ Wo + bo

Sharding: data-parallel over the 16384 token rows -> 2048 rows per core.
Each core runs the same Bass program on its row slab with all weights
resident in SBUF.

Kernel design (per 128-token tile, token-major):
  - x^T obtained via 8 PE transposes (f32r; inputs get rounded to f32r by
    the matmul anyway)
  - q/k/v projections as float32r matmuls (full rate at N=512, ~1.6e-4
    rounding vs 2.3e-3 for bf16); q/k/v biases folded in via K=1 ones-row
    matmuls, the output bias via a broadcast tensor_add during evacuation
  - feature map on ScalarE (Square, then Exp with scale=-0.5; both live in
    the exp_and_others ACT table set -> one table load)
  - per-head reduction + coeff*v broadcast on VectorE (step-0 free-dim AP)
  - qkv transposed back via PE, output projection + bias, ScalarE evac, DMA
  - two-stage software pipeline: tile t's transposes+projections are
    emitted before tile t-1's elementwise+output stage so the PE never
    waits on the ACT/DVE chain
  - DMAs split across queues: x tiles + Wk/Wv chunks on the Scalar HWDGE
    queue, Wq/biases/Wo on Sync, so the first tiles start immediately
"""
import numpy as np
import concourse.bass as bass
import concourse.tile as tile
from concourse import bacc, mybir
import concourse.bass_utils as bass_utils
from concourse.masks import make_identity

F32 = mybir.dt.float32
F32R = mybir.dt.float32r

B, S, D = 4, 4096, 1024
H = 16
HD = D // H
NCORES = 8
ROWS = B * S // NCORES  # 2048 rows per core
NCHUNK = D // 128  # 8 contraction chunks


def _build(rows=ROWS, num_devices=NCORES):
    ntiles = rows // 128
    nc = bacc.Bacc("TRN2", target_bir_lowering=False, debug=False,
                   num_devices=num_devices)
    X = nc.dram_tensor("x", [rows, D], F32R, kind="ExternalInput").ap()
    Wq = nc.dram_tensor("Wq", [D, D], F32R, kind="ExternalInput").ap()
    Wk = nc.dram_tensor("Wk", [D, D], F32R, kind="ExternalInput").ap()
    Wv = nc.dram_tensor("Wv", [D, D], F32R, kind="ExternalInput").ap()
    Wo = nc.dram_tensor("Wo", [D, D], F32R, kind="ExternalInput").ap()
    BQ = nc.dram_tensor("bq", [D], F32R, kind="ExternalInput").ap()
    BK = nc.dram_tensor("bk", [D], F32R, kind="ExternalInput").ap()
    BV = nc.dram_tensor("bv", [D], F32R, kind="ExternalInput").ap()
    BO = nc.dram_tensor("bo", [D], F32R, kind="ExternalInput").ap()
    OUT = nc.dram_tensor("out", [rows, D], F32, kind="ExternalOutput").ap()

    with tile.TileContext(nc) as tc:
        with tc.tile_pool(name="wpool", bufs=1) as wpool, \
             tc.tile_pool(name="cpool", bufs=1) as cpool, \
             tc.tile_pool(name="xpool", bufs=3) as xpool, \
             tc.tile_pool(name="xtp", bufs=2) as xtp, \
             tc.tile_pool(name="ew", bufs=2) as ew, \
             tc.tile_pool(name="qkvp", bufs=2) as qkvp, \
             tc.tile_pool(name="outp", bufs=2) as outp, \
             tc.tile_pool(name="ps", bufs=8, space="PSUM") as ps:

            # ---- one-time setup ----
            ident_f32 = cpool.tile([128, 128], F32, tag="ident_f32")
            make_identity(nc, ident_f32)
            ident = cpool.tile([128, 128], F32R, tag="ident")
            nc.vector.tensor_copy(ident, ident_f32)

            # broadcast bias tiles: b_bc[w][p, :] = bias of weight w
            b_bc = [cpool.tile([128, D], F32R, tag=f"b_bc{w}", name=f"b_bc{w}")
                    for w in range(4)]
            bo_bc = b_bc[3]

            # weights: w_sb[w] is [128, NCHUNK, D]; chunk c = W[c*128:(c+1)*128]
            w_sb = []
            for w in range(4):
                t = wpool.tile([128, NCHUNK, D], F32R, tag=f"w{w}",
                               name=f"w{w}")
                w_sb.append(t)
            for c in range(NCHUNK):
                nc.sync.dma_start(out=w_sb[0][:, c, :],
                                  in_=Wq[c * 128:(c + 1) * 128, :])
            nc.sync.dma_start(
                out=b_bc[0],
                in_=bass.AP(tensor=BQ.tensor, offset=BQ.offset,
                            ap=[[0, 128], [BQ.ap[0][0], D]]))
            for c in range(NCHUNK):
                nc.sync.dma_start(out=w_sb[3][:, c, :],
                                  in_=Wo[c * 128:(c + 1) * 128, :])
            nc.sync.dma_start(
                out=b_bc[3],
                in_=bass.AP(tensor=BO.tensor, offset=BO.offset,
                            ap=[[0, 128], [BO.ap[0][0], D]]))

            def dma_weights_kv():
                for c in range(NCHUNK):
                    for w, W in [(1, Wk), (2, Wv)]:
                        nc.scalar.dma_start(out=w_sb[w][:, c, :],
                                            in_=W[c * 128:(c + 1) * 128, :])
                for w, Bi in [(1, BK), (2, BV)]:
                    nc.scalar.dma_start(
                        out=b_bc[w],
                        in_=bass.AP(tensor=Bi.tensor, offset=Bi.offset,
                                    ap=[[0, 128], [Bi.ap[0][0], D]]))

            x_tiles = [None] * ntiles

            def dma_x(t):
                x_sb = xpool.tile([128, D], F32R, tag="x", name="x_sb")
                nc.scalar.dma_start(out=x_sb, in_=X[t * 128:(t + 1) * 128, :])
                x_tiles[t] = x_sb

            early = [None] * ntiles  # t -> (proj_ps[3],)
            xTs = [None] * ntiles

            def stage_transpose(t):
                x_sb = x_tiles[t]
                xT = xtp.tile([128, NCHUNK, 128], F32R, tag="xT", name="xT")
                xTf = xT.rearrange("p c t -> p (c t)")
                for g in range(2):
                    xT_ps = ps.tile([128, 512], F32R, tag="ps", name="xT_ps")
                    for cc in range(4):
                        c = g * 4 + cc
                        nc.tensor.transpose(
                            xT_ps[:, cc * 128:(cc + 1) * 128],
                            x_sb[:, c * 128:(c + 1) * 128], ident)
                    nc.vector.tensor_copy(xTf[:, g * 512:(g + 1) * 512], xT_ps)
                xTs[t] = xT

            def stage_proj(t):
                xT = xTs[t]
                proj_ps = [[ps.tile([128, 512], F32, tag="ps",
                                    name=f"proj{w}h{h}")
                            for h in range(2)] for w in range(3)]
                for c in range(NCHUNK):
                    for w in range(3):
                        for h in range(2):
                            nc.tensor.matmul(
                                proj_ps[w][h],
                                xT[:, c, :],
                                w_sb[w][:, c, h * 512:(h + 1) * 512],
                                start=(c == 0), stop=(c == NCHUNK - 1))
                early[t] = proj_ps

            qkvTs = [None] * ntiles
            qkvs = [None] * ntiles

            def stage_mid(t):
                stage_mid_ew(t)
                stage_mid_T(t)

            def stage_mid_ew(t):
                proj_ps = early[t]
                # feature map: qe = exp(-0.5*(q+b)^2), ke likewise.
                # bias add on VectorE (broadcast tile), square+exp on ScalarE
                qb = ew.tile([128, D], F32, tag="sq", name="qb")
                for h in range(2):
                    nc.vector.tensor_add(qb[:, h * 512:(h + 1) * 512],
                                         proj_ps[0][h],
                                         b_bc[0][:, h * 512:(h + 1) * 512])
                sq_q = ew.tile([128, D], F32, tag="sq", name="sq_q")
                nc.scalar.activation(sq_q, qb,
                                     mybir.ActivationFunctionType.Square)
                qe = ew.tile([128, D], F32, tag="qe", name="qe")
                nc.scalar.activation(qe, sq_q,
                                     mybir.ActivationFunctionType.Exp,
                                     scale=-0.5)
                kb = ew.tile([128, D], F32, tag="sq", name="kb")
                for h in range(2):
                    nc.vector.tensor_add(kb[:, h * 512:(h + 1) * 512],
                                         proj_ps[1][h],
                                         b_bc[1][:, h * 512:(h + 1) * 512])
                sq_k = ew.tile([128, D], F32, tag="sq", name="sq_k")
                nc.scalar.activation(sq_k, kb,
                                     mybir.ActivationFunctionType.Square)
                ke = ew.tile([128, D], F32, tag="qe", name="ke")
                nc.scalar.activation(ke, sq_k,
                                     mybir.ActivationFunctionType.Exp,
                                     scale=-0.5)

                # coeff[p, h] = sum_j qe[p,h,j]*ke[p,h,j]
                qk = ew.tile([128, D], F32, tag="sq", name="qk")
                nc.vector.tensor_mul(qk, qe, ke)
                coeff = ew.tile([128, H], F32, tag="coeff", name="coeff")
                nc.vector.reduce_sum(coeff,
                                     qk.rearrange("p (h j) -> p h j", h=H),
                                     axis=mybir.AxisListType.X)

                # qkv = coeff (broadcast over HD) * (v + bv) -> f32r
                vb = ew.tile([128, D], F32, tag="sq", name="vb")
                for h in range(2):
                    nc.vector.tensor_add(vb[:, h * 512:(h + 1) * 512],
                                         proj_ps[2][h],
                                         b_bc[2][:, h * 512:(h + 1) * 512])
                qkv = qkvp.tile([128, D], F32R, tag="qkv", name="qkv")
                coeff_b = bass.AP(
                    tensor=coeff.tensor, offset=coeff.offset,
                    ap=[coeff.ap[0], [coeff.ap[1][0], H], [0, HD]])
                nc.vector.tensor_mul(
                    qkv.rearrange("p (h j) -> p h j", h=H),
                    vb.rearrange("p (h j) -> p h j", h=H),
                    coeff_b)

                qkvs[t] = qkv

            def stage_mid_T(t):
                qkv = qkvs[t]
                # transpose qkv
                qkvT = qkvp.tile([128, NCHUNK, 128], F32R, tag="qkvT",
                                 name="qkvT")
                qkvTf = qkvT.rearrange("p c t -> p (c t)")
                for g in range(2):
                    qkvT_ps = ps.tile([128, 512], F32R, tag="ps",
                                      name="qkvT_ps")
                    for cc in range(4):
                        c = g * 4 + cc
                        nc.tensor.transpose(
                            qkvT_ps[:, cc * 128:(cc + 1) * 128],
                            qkv[:, c * 128:(c + 1) * 128], ident)
                    nc.scalar.copy(qkvTf[:, g * 512:(g + 1) * 512], qkvT_ps)
                qkvTs[t] = qkvT

            def stage_out(t):
                rs = t * 128
                qkvT = qkvTs[t]
                # output projection + bias
                out_ps = [ps.tile([128, 512], F32, tag="ps", name=f"out_h{h}")
                          for h in range(2)]
                for c in range(NCHUNK):
                    for h in range(2):
                        nc.tensor.matmul(
                            out_ps[h],
                            qkvT[:, c, :],
                            w_sb[3][:, c, h * 512:(h + 1) * 512],
                            start=(c == 0), stop=(c == NCHUNK - 1))
                out_sb = outp.tile([128, D], F32, tag="out", name="out_sb")
                for h in range(2):
                    nc.vector.tensor_add(out_sb[:, h * 512:(h + 1) * 512],
                                         out_ps[h],
                                         bo_bc[:, h * 512:(h + 1) * 512])
                nc.scalar.dma_start(out=OUT[rs:rs + 128, :], in_=out_sb)

            # ---- pipelined emission (transpose one tile ahead) ----
            dma_x(0)
            if ntiles > 1:
                dma_x(1)
            dma_weights_kv()
            stage_transpose(0)
            for t in range(ntiles):
                if t + 2 < ntiles:
                    dma_x(t + 2)
                if t + 1 < ntiles:
                    stage_transpose(t + 1)
                stage_proj(t)
                if t >= 2:
                    stage_out(t - 2)
                if t >= 1:
                    stage_mid(t - 1)
            stage_mid_ew(ntiles - 1)
            if ntiles >= 2:
                stage_out(ntiles - 2)
            stage_mid_T(ntiles - 1)
            stage_out(ntiles - 1)

    nc.compile()
    return nc


_NC = None
LAST_RESULT = None  # BassKernelResults of the most recent run (for test.py)


def kernel(x, Wq, bq, Wk, bk, Wv, bv, Wo, bo, _trace=False):
    global _NC, LAST_RESULT
    if _NC is None:
        _NC = _build()

    x = np.ascontiguousarray(np.asarray(x, dtype=np.float32)).reshape(B * S, D)
    weights = {
        "Wq": np.ascontiguousarray(np.asarray(Wq, dtype=np.float32)),
        "Wk": np.ascontiguousarray(np.asarray(Wk, dtype=np.float32)),
        "Wv": np.ascontiguousarray(np.asarray(Wv, dtype=np.float32)),
        "Wo": np.ascontiguousarray(np.asarray(Wo, dtype=np.float32)),
        "bq": np.ascontiguousarray(np.asarray(bq, dtype=np.float32)),
        "bk": np.ascontiguousarray(np.asarray(bk, dtype=np.float32)),
        "bv": np.ascontiguousarray(np.asarray(bv, dtype=np.float32)),
        "bo": np.ascontiguousarray(np.asarray(bo, dtype=np.float32)),
    }
    in_maps = [{"x": x[i * ROWS:(i + 1) * ROWS], **weights}
               for i in range(NCORES)]
    res = bass_utils.run_bass_kernel_spmd(
        _NC, in_maps, core_ids=list(range(NCORES)), trace=_trace)
    LAST_RESULT = res
    out = np.concatenate([res.results[i]["out"] for i in range(NCORES)],
                         axis=0)
    return out.reshape(B, S, D).astype(np.float32)
